# revision 16
# baseline (speedup 1.0000x reference)
"""RNN-T joint network kernel for 8 Trainium2 NeuronCores.

out[b,t,u,:] = relu(enc_proj[b,t,:] + dec_proj[b,u,:]) @ W_out + b_out

Sharding: data-parallel over (b, t-half): core i = (b = i//2, t_half = i%2)
owns output rows [i*10000, (i+1)*10000) of the flattened [80000, 1025]
output -- a contiguous 41 MB slab per core, gathered by simple concat.

Per-core plan:
  phase 1 (fp32 PE): enc_projT [H,100] and dec_projT [H,100] with bias
  phase 2: 80 chunks of 125 (t,u) rows:
    - DVE builds xT[k] = relu(decT[k][:,u-run] + encT[k][:,t]) per 128-row
      h-chunk (tensor_scalar add+max, per-partition scalar = enc column)
    - PE (float32r): psum[125, vtile] = b_out (K=1 ones matmul) then
      accumulates 5 K-chunks vs streamed W_out; V tiled {342,342,341}
    - ACT copies PSUM->SBUF, one 512 KB DMA per chunk to HBM
"""

import sys

if "/opt/trn_rl_repo" not in sys.path:
    sys.path.insert(0, "/opt/trn_rl_repo")

import numpy as np

B, D_ENC, T = 4, 512, 200
D_DEC, U = 640, 100
H, V = 640, 1025
VP = 1026                # V padded: f32r matmul needs even free counts
N_CORES = 8
TC = T // 2              # 100 t values per core
ROWS = TC * U            # 10000 output rows per core
CH = 128                 # rows per chunk (stationary M, must be even)
KE = D_ENC // 128        # 4
KD = D_DEC // 128        # 5
KH = H // 128            # 5
V_TILES = [(0, 342), (342, 342), (684, 342)]
# (row0, nrows) chunks: 78 full 128-row chunks + one 16-row tail
CHUNKS = [(i * CH, min(CH, ROWS - i * CH))
          for i in range((ROWS + CH - 1) // CH)]

_CACHE = {}


def _build_program():
    import concourse.tile as tile
    from concourse import bacc, mybir

    f32 = mybir.dt.float32
    f32r = mybir.dt.float32r
    ADD = mybir.AluOpType.add
    MAX = mybir.AluOpType.max

    nc = bacc.Bacc("TRN2", target_bir_lowering=False, debug=False,
                   num_devices=N_CORES)

    enc_d = nc.dram_tensor("enc", [D_ENC, TC], f32, kind="ExternalInput").ap()
    dec_d = nc.dram_tensor("dec", [D_DEC, U], f32, kind="ExternalInput").ap()
    w_enc_d = nc.dram_tensor("w_enc", [D_ENC, H], f32, kind="ExternalInput").ap()
    w_pred_d = nc.dram_tensor("w_pred", [D_DEC, H], f32, kind="ExternalInput").ap()
    w_out_d = nc.dram_tensor("w_out", [H, VP], f32, kind="ExternalInput").ap()
    b_enc_d = nc.dram_tensor("b_enc", [H, 1], f32, kind="ExternalInput").ap()
    b_pred_d = nc.dram_tensor("b_pred", [H, 1], f32, kind="ExternalInput").ap()
    b_out_d = nc.dram_tensor("b_out", [1, VP], f32, kind="ExternalInput").ap()
    ones_d = nc.dram_tensor("ones", [1, CH], f32, kind="ExternalInput").ap()
    out_d = nc.dram_tensor("out", [ROWS, V], f32, kind="ExternalOutput").ap()

    with tile.TileContext(nc) as tc:
        with (
            tc.tile_pool(name="const", bufs=1) as cpool,
            tc.tile_pool(name="xp", bufs=3) as xpool,
            tc.tile_pool(name="op", bufs=3) as opool,
            tc.tile_pool(name="pp", bufs=2, space="PSUM") as ppool,
        ):
            # ---- load inputs -------------------------------------------------
            w_enc_sb = []
            for d in range(KE):
                t_ = cpool.tile([128, H], f32, tag=f"we{d}", name=f"we{d}")
                nc.sync.dma_start(t_[:], w_enc_d[d * 128:(d + 1) * 128, :])
                w_enc_sb.append(t_)
            w_pred_sb = []
            for d in range(KD):
                t_ = cpool.tile([128, H], f32, tag=f"wp{d}", name=f"wp{d}")
                nc.sync.dma_start(t_[:], w_pred_d[d * 128:(d + 1) * 128, :])
                w_pred_sb.append(t_)
            w_out_sb = []
            for k in range(KH):
                t_ = cpool.tile([128, VP], f32r, tag=f"wo{k}", name=f"wo{k}")
                nc.sync.dma_start(
                    t_[:], w_out_d[k * 128:(k + 1) * 128, :].bitcast(f32r))
                w_out_sb.append(t_)
            enc_sb = []
            for d in range(KE):
                t_ = cpool.tile([128, TC], f32, tag=f"es{d}", name=f"es{d}")
                nc.sync.dma_start(t_[:], enc_d[d * 128:(d + 1) * 128, :])
                enc_sb.append(t_)
            dec_sb = []
            for d in range(KD):
                t_ = cpool.tile([128, U], f32, tag=f"ds{d}", name=f"ds{d}")
                nc.sync.dma_start(t_[:], dec_d[d * 128:(d + 1) * 128, :])
                dec_sb.append(t_)
            b_enc_sb = []
            b_pred_sb = []
            for k in range(KH):
                t_ = cpool.tile([128, 1], f32, tag=f"be{k}", name=f"be{k}")
                nc.sync.dma_start(t_[:], b_enc_d[k * 128:(k + 1) * 128, :])
                b_enc_sb.append(t_)
                t_ = cpool.tile([128, 1], f32, tag=f"bp{k}", name=f"bp{k}")
                nc.sync.dma_start(t_[:], b_pred_d[k * 128:(k + 1) * 128, :])
                b_pred_sb.append(t_)
            bout_row = cpool.tile([1, VP], f32r, tag="bo", name="bo")
            nc.sync.dma_start(bout_row[:], b_out_d[0:1, :].bitcast(f32r))
            ones_row = cpool.tile([1, CH], f32r, tag="ones", name="ones")
            nc.sync.dma_start(ones_row[:], ones_d[0:1, :].bitcast(f32r))

            # ---- phase 1: projections (full fp32) ---------------------------
            encT = []
            decT = []
            for k in range(KH):
                pe_ = ppool.tile([128, TC], f32, tag="pp1", name=f"pe{k}")
                for d in range(KE):
                    nc.tensor.matmul(
                        pe_[:], w_enc_sb[d][:, k * 128:(k + 1) * 128],
                        enc_sb[d][:], start=(d == 0), stop=(d == KE - 1))
                et = cpool.tile([128, TC], f32, tag=f"et{k}", name=f"et{k}")
                nc.scalar.add(et[:], pe_[:], b_enc_sb[k][:, 0:1])
                encT.append(et)

                pd_ = ppool.tile([128, U], f32, tag="pp1", name=f"pd{k}")
                for d in range(KD):
                    nc.tensor.matmul(
                        pd_[:], w_pred_sb[d][:, k * 128:(k + 1) * 128],
                        dec_sb[d][:], start=(d == 0), stop=(d == KD - 1))
                dtl = cpool.tile([128, U], f32, tag=f"dt{k}", name=f"dt{k}")
                nc.scalar.add(dtl[:], pd_[:], b_pred_sb[k][:, 0:1])
                decT.append(dtl)

            ones_r = ones_row[0:1, 0:CH]
            bout_r = bout_row[0:1, :]

            # ---- phase 2: joint + output matmul -----------------------------
            for c, (r0, m) in enumerate(CHUNKS):
                runs = []
                r = r0
                while r < r0 + m:
                    tt, uu = divmod(r, U)
                    ln = min(U - uu, r0 + m - r)
                    runs.append((tt, uu, ln))
                    r += ln

                xts = []
                for k in range(KH):
                    xt = xpool.tile([128, CH], f32r, tag=f"x{k}",
                                    name=f"x{k}_{c}")
                    off = 0
                    for (tt, uu, ln) in runs:
                        nc.vector.tensor_scalar(
                            out=xt[:, off:off + ln],
                            in0=decT[k][:, uu:uu + ln],
                            scalar1=encT[k][:, tt:tt + 1],
                            scalar2=0.0, op0=ADD, op1=MAX)
                        off += ln
                    xts.append(xt)

                psums = []
                for vi, (v0, vw) in enumerate(V_TILES):
                    ps = ppool.tile([m, vw], f32, tag=f"p{vi}",
                                    name=f"p{vi}_{c}")
                    nc.tensor.matmul(ps[:], ones_r[:, 0:m],
                                     bout_r[:, v0:v0 + vw],
                                     start=True, stop=False)
                    psums.append(ps)
                for k in range(KH):
                    lhs = xts[k][:, 0:m]
                    for vi, (v0, vw) in enumerate(V_TILES):
                        nc.tensor.matmul(
                            psums[vi][:], lhs,
                            w_out_sb[k][:, v0:v0 + vw],
                            start=False, stop=(k == KH - 1))

                osb = opool.tile([128, VP], f32, tag="osb", name=f"o{c}")
                for vi, (v0, vw) in enumerate(V_TILES):
                    nc.scalar.copy(osb[0:m, v0:v0 + vw], psums[vi][:])
                nc.sync.dma_start(out_d[r0:r0 + m, :], osb[0:m, 0:V])

    nc.compile()
    return nc


def make_in_maps(inputs):
    enc = np.ascontiguousarray(np.asarray(inputs["encoder_outputs"], np.float32))
    dec = np.ascontiguousarray(np.asarray(inputs["decoder_outputs"], np.float32))
    w_enc = np.ascontiguousarray(np.asarray(inputs["W_enc"], np.float32))
    w_pred = np.ascontiguousarray(np.asarray(inputs["W_pred"], np.float32))
    w_out = np.zeros((H, VP), np.float32)
    w_out[:, :V] = np.asarray(inputs["W_out"], np.float32)
    b_enc = np.asarray(inputs["b_enc"], np.float32).reshape(H, 1)
    b_pred = np.asarray(inputs["b_pred"], np.float32).reshape(H, 1)
    b_out = np.zeros((1, VP), np.float32)
    b_out[0, :V] = np.asarray(inputs["b_out"], np.float32)

    in_maps = []
    for i in range(N_CORES):
        b, th = divmod(i, 2)
        in_maps.append({
            "enc": np.ascontiguousarray(enc[b, :, th * TC:(th + 1) * TC]),
            "dec": dec[b],
            "w_enc": w_enc,
            "w_pred": w_pred,
            "w_out": w_out,
            "b_enc": b_enc,
            "b_pred": b_pred,
            "b_out": b_out,
            "ones": np.ones((1, CH), np.float32),
        })
    return in_maps


def kernel(**inputs):
    from concourse.bass_utils import run_bass_kernel_spmd

    if "nc" not in _CACHE:
        _CACHE["nc"] = _build_program()
    nc = _CACHE["nc"]

    res = run_bass_kernel_spmd(nc, make_in_maps(inputs), list(range(N_CORES)))
    outs = np.stack([res.results[i]["out"] for i in range(N_CORES)])
    return outs.reshape(B, T, U, V).astype(np.float32, copy=False)


# revision 23
# speedup vs baseline: 1.1580x; 1.1580x over previous
"""RNN-T joint network kernel for 8 Trainium2 NeuronCores.

out[b,t,u,:] = relu(enc_proj[b,t,:] + dec_proj[b,u,:]) @ W_out + b_out

Sharding: data-parallel over (b, t-half): core i = (b = i//2, t_half = i%2)
owns output rows [i*10000, (i+1)*10000) of the flattened [80000, 1025]
output -- a contiguous 41 MB slab per core, gathered by simple concat.

Per-core plan:
  phase 1 (fp32 PE): enc_projT [H,100] and dec_projT [H,100] with bias
  phase 2: 80 chunks of 125 (t,u) rows:
    - DVE builds xT[k] = relu(decT[k][:,u-run] + encT[k][:,t]) per 128-row
      h-chunk (tensor_scalar add+max, per-partition scalar = enc column)
    - PE (float32r): psum[125, vtile] = b_out (K=1 ones matmul) then
      accumulates 5 K-chunks vs streamed W_out; V tiled {342,342,341}
    - ACT copies PSUM->SBUF, one 512 KB DMA per chunk to HBM
"""

import sys

if "/opt/trn_rl_repo" not in sys.path:
    sys.path.insert(0, "/opt/trn_rl_repo")

import numpy as np

B, D_ENC, T = 4, 512, 200
D_DEC, U = 640, 100
H, V = 640, 1025
VP = 1026                # V padded: f32r matmul needs even free counts
N_CORES = 8
TC = T // 2              # 100 t values per core
ROWS = TC * U            # 10000 output rows per core
CH = 128                 # rows per chunk (stationary M, must be even)
KE = D_ENC // 128        # 4
KD = D_DEC // 128        # 5
KH = H // 128            # 5
V_TILES = [(0, 342), (342, 342), (684, 342)]
GT = 20                  # t values per X-group tile
NG = TC // GT            # 5 groups
GROWS = GT * U           # 2000 rows per group
# chunk offsets within a group: 15 x 128 + one 80-row tail (all even)
GCHUNKS = [(i * CH, min(CH, GROWS - i * CH))
           for i in range((GROWS + CH - 1) // CH)]

_CACHE = {}


def _build_program():
    import concourse.tile as tile
    from concourse import bacc, mybir

    f32 = mybir.dt.float32
    f32r = mybir.dt.float32r

    nc = bacc.Bacc("TRN2", target_bir_lowering=False, debug=False,
                   num_devices=N_CORES)

    enc_d = nc.dram_tensor("enc", [D_ENC, TC], f32, kind="ExternalInput").ap()
    dec_d = nc.dram_tensor("dec", [D_DEC, U], f32, kind="ExternalInput").ap()
    w_enc_d = nc.dram_tensor("w_enc", [D_ENC, H], f32, kind="ExternalInput").ap()
    w_pred_d = nc.dram_tensor("w_pred", [D_DEC, H], f32, kind="ExternalInput").ap()
    w_out_d = nc.dram_tensor("w_out", [H, VP], f32, kind="ExternalInput").ap()
    b_enc_d = nc.dram_tensor("b_enc", [H, 1], f32, kind="ExternalInput").ap()
    b_pred_d = nc.dram_tensor("b_pred", [H, 1], f32, kind="ExternalInput").ap()
    # b_out replicated across 128 partitions on the host
    b_out_d = nc.dram_tensor("b_out", [128, VP], f32, kind="ExternalInput").ap()
    out_d = nc.dram_tensor("out", [ROWS, V], f32, kind="ExternalOutput").ap()

    with tile.TileContext(nc) as tc:
        with (
            tc.tile_pool(name="const", bufs=1) as cpool,
            tc.tile_pool(name="xp", bufs=3) as xpool,
            tc.tile_pool(name="op", bufs=3) as opool,
            tc.tile_pool(name="pp", bufs=2, space="PSUM") as ppool,
        ):
            # ---- load inputs -------------------------------------------------
            w_enc_sb = []
            for d in range(KE):
                t_ = cpool.tile([128, H], f32, tag=f"we{d}", name=f"we{d}")
                nc.sync.dma_start(t_[:], w_enc_d[d * 128:(d + 1) * 128, :])
                w_enc_sb.append(t_)
            w_pred_sb = []
            for d in range(KD):
                t_ = cpool.tile([128, H], f32, tag=f"wp{d}", name=f"wp{d}")
                nc.sync.dma_start(t_[:], w_pred_d[d * 128:(d + 1) * 128, :])
                w_pred_sb.append(t_)
            w_out_sb = []
            for k in range(KH):
                t_ = cpool.tile([128, VP], f32r, tag=f"wo{k}", name=f"wo{k}")
                nc.sync.dma_start(
                    t_[:], w_out_d[k * 128:(k + 1) * 128, :].bitcast(f32r))
                w_out_sb.append(t_)
            enc_sb = []
            for d in range(KE):
                t_ = cpool.tile([128, TC], f32, tag=f"es{d}", name=f"es{d}")
                nc.sync.dma_start(t_[:], enc_d[d * 128:(d + 1) * 128, :])
                enc_sb.append(t_)
            dec_sb = []
            for d in range(KD):
                t_ = cpool.tile([128, U], f32, tag=f"ds{d}", name=f"ds{d}")
                nc.sync.dma_start(t_[:], dec_d[d * 128:(d + 1) * 128, :])
                dec_sb.append(t_)
            b_enc_sb = []
            b_pred_sb = []
            for k in range(KH):
                t_ = cpool.tile([128, 1], f32, tag=f"be{k}", name=f"be{k}")
                nc.sync.dma_start(t_[:], b_enc_d[k * 128:(k + 1) * 128, :])
                b_enc_sb.append(t_)
                t_ = cpool.tile([128, 1], f32, tag=f"bp{k}", name=f"bp{k}")
                nc.sync.dma_start(t_[:], b_pred_d[k * 128:(k + 1) * 128, :])
                b_pred_sb.append(t_)
            bout_rep = cpool.tile([128, VP], f32, tag="bo", name="bo")
            nc.sync.dma_start(bout_rep[:], b_out_d[:, :])

            # ---- phase 1: projections (full fp32) ---------------------------
            encT = []
            decT = []
            for k in range(KH):
                pe_ = ppool.tile([128, TC], f32, tag="pp1", name=f"pe{k}")
                for d in range(KE):
                    nc.tensor.matmul(
                        pe_[:], w_enc_sb[d][:, k * 128:(k + 1) * 128],
                        enc_sb[d][:], start=(d == 0), stop=(d == KE - 1))
                et = cpool.tile([128, TC], f32, tag=f"et{k}", name=f"et{k}")
                nc.scalar.add(et[:], pe_[:], b_enc_sb[k][:, 0:1])
                encT.append(et)

                pd_ = ppool.tile([128, U], f32, tag="pp1", name=f"pd{k}")
                for d in range(KD):
                    nc.tensor.matmul(
                        pd_[:], w_pred_sb[d][:, k * 128:(k + 1) * 128],
                        dec_sb[d][:], start=(d == 0), stop=(d == KD - 1))
                dtl = cpool.tile([128, U], f32, tag=f"dt{k}", name=f"dt{k}")
                nc.scalar.add(dtl[:], pd_[:], b_pred_sb[k][:, 0:1])
                decT.append(dtl)

            # ---- phase 2: joint + output matmul -----------------------------
            Relu = mybir.ActivationFunctionType.Relu
            for g in range(NG):
                # build X^T for 20 t values: one ACT Relu+bias per (t, k)
                xbs = []
                for k in range(KH):
                    xb = xpool.tile([128, GROWS], f32r, tag=f"xb{k}",
                                    name=f"xb{k}_{g}")
                    for tl in range(GT):
                        t = g * GT + tl
                        nc.scalar.activation(
                            xb[:, tl * U:(tl + 1) * U], decT[k][:],
                            Relu, bias=encT[k][:, t:t + 1], scale=1.0)
                    xbs.append(xb)

                for ci, (off, m) in enumerate(GCHUNKS):
                    r0 = g * GROWS + off
                    psums = []
                    for vi, (v0, vw) in enumerate(V_TILES):
                        ps = ppool.tile([m, vw], f32, tag=f"p{vi}",
                                        name=f"p{vi}_{g}_{ci}")
                        psums.append(ps)
                    for k in range(KH):
                        lhs = xbs[k][:, off:off + m]
                        for vi, (v0, vw) in enumerate(V_TILES):
                            nc.tensor.matmul(
                                psums[vi][:], lhs,
                                w_out_sb[k][:, v0:v0 + vw],
                                start=(k == 0), stop=(k == KH - 1))

                    osb = opool.tile([128, VP], f32, tag="osb",
                                     name=f"o{g}_{ci}")
                    for vi, (v0, vw) in enumerate(V_TILES):
                        nc.vector.tensor_add(
                            osb[0:m, v0:v0 + vw], psums[vi][:],
                            bout_rep[0:m, v0:v0 + vw])
                    nc.sync.dma_start(out_d[r0:r0 + m, :], osb[0:m, 0:V])

    nc.compile()
    return nc


def make_in_maps(inputs):
    enc = np.ascontiguousarray(np.asarray(inputs["encoder_outputs"], np.float32))
    dec = np.ascontiguousarray(np.asarray(inputs["decoder_outputs"], np.float32))
    w_enc = np.ascontiguousarray(np.asarray(inputs["W_enc"], np.float32))
    w_pred = np.ascontiguousarray(np.asarray(inputs["W_pred"], np.float32))
    w_out = np.zeros((H, VP), np.float32)
    w_out[:, :V] = np.asarray(inputs["W_out"], np.float32)
    b_enc = np.asarray(inputs["b_enc"], np.float32).reshape(H, 1)
    b_pred = np.asarray(inputs["b_pred"], np.float32).reshape(H, 1)
    b_out = np.zeros((1, VP), np.float32)
    b_out[0, :V] = np.asarray(inputs["b_out"], np.float32)
    b_out_rep = np.ascontiguousarray(np.broadcast_to(b_out, (128, VP)))

    in_maps = []
    for i in range(N_CORES):
        b, th = divmod(i, 2)
        in_maps.append({
            "enc": np.ascontiguousarray(enc[b, :, th * TC:(th + 1) * TC]),
            "dec": dec[b],
            "w_enc": w_enc,
            "w_pred": w_pred,
            "w_out": w_out,
            "b_enc": b_enc,
            "b_pred": b_pred,
            "b_out": b_out_rep,
        })
    return in_maps


def kernel(**inputs):
    from concourse.bass_utils import run_bass_kernel_spmd

    if "nc" not in _CACHE:
        _CACHE["nc"] = _build_program()
    nc = _CACHE["nc"]

    res = run_bass_kernel_spmd(nc, make_in_maps(inputs), list(range(N_CORES)))
    outs = np.stack([res.results[i]["out"] for i in range(N_CORES)])
    return outs.reshape(B, T, U, V).astype(np.float32, copy=False)


# revision 28
# speedup vs baseline: 1.2605x; 1.0885x over previous
"""RNN-T joint network kernel for 8 Trainium2 NeuronCores.

out[b,t,u,:] = relu(enc_proj[b,t,:] + dec_proj[b,u,:]) @ W_out + b_out

Sharding: data-parallel over (b, t-half): core i = (b = i//2, t_half = i%2)
owns output rows [i*10000, (i+1)*10000) of the flattened [80000, 1025]
output -- a contiguous 41 MB slab per core, gathered by simple concat.

Per-core plan:
  phase 1 (fp32 PE): enc_projT [H,100] and dec_projT [H,100] with bias
  phase 2: 80 chunks of 125 (t,u) rows:
    - DVE builds xT[k] = relu(decT[k][:,u-run] + encT[k][:,t]) per 128-row
      h-chunk (tensor_scalar add+max, per-partition scalar = enc column)
    - PE (float32r): psum[125, vtile] = b_out (K=1 ones matmul) then
      accumulates 5 K-chunks vs streamed W_out; V tiled {342,342,341}
    - ACT copies PSUM->SBUF, one 512 KB DMA per chunk to HBM
"""

import sys

if "/opt/trn_rl_repo" not in sys.path:
    sys.path.insert(0, "/opt/trn_rl_repo")

import numpy as np

B, D_ENC, T = 4, 512, 200
D_DEC, U = 640, 100
H, V = 640, 1025
VP = 1026                # V padded: f32r matmul needs even free counts
N_CORES = 8
TC = T // 2              # 100 t values per core
ROWS = TC * U            # 10000 output rows per core
CH = 128                 # rows per chunk (stationary M, must be even)
KE = D_ENC // 128        # 4
KD = D_DEC // 128        # 5
KH = H // 128            # 5
V_TILES = [(0, 342), (342, 342), (684, 342)]
GT = 20                  # t values per X-group tile
NG = TC // GT            # 5 groups
GROWS = GT * U           # 2000 rows per group
# chunk offsets within a group: 15 x 128 + one 80-row tail (all even)
GCHUNKS = [(i * CH, min(CH, GROWS - i * CH))
           for i in range((GROWS + CH - 1) // CH)]

_CACHE = {}


def _build_program():
    import concourse.tile as tile
    from concourse import bacc, mybir

    f32 = mybir.dt.float32
    bf16 = mybir.dt.bfloat16

    nc = bacc.Bacc("TRN2", target_bir_lowering=False, debug=False,
                   num_devices=N_CORES)

    enc_d = nc.dram_tensor("enc", [D_ENC, TC], f32, kind="ExternalInput").ap()
    dec_d = nc.dram_tensor("dec", [D_DEC, U], f32, kind="ExternalInput").ap()
    w_enc_d = nc.dram_tensor("w_enc", [D_ENC, H], f32, kind="ExternalInput").ap()
    w_pred_d = nc.dram_tensor("w_pred", [D_DEC, H], f32, kind="ExternalInput").ap()
    w_out_d = nc.dram_tensor("w_out", [H, VP], bf16, kind="ExternalInput").ap()
    b_enc_d = nc.dram_tensor("b_enc", [H, 1], f32, kind="ExternalInput").ap()
    b_pred_d = nc.dram_tensor("b_pred", [H, 1], f32, kind="ExternalInput").ap()
    # b_out replicated across 128 partitions on the host
    b_out_d = nc.dram_tensor("b_out", [128, VP], f32, kind="ExternalInput").ap()
    out_d = nc.dram_tensor("out", [ROWS, V], f32, kind="ExternalOutput").ap()

    with tile.TileContext(nc) as tc:
        with (
            tc.tile_pool(name="const", bufs=1) as cpool,
            tc.tile_pool(name="xp", bufs=3) as xpool,
            tc.tile_pool(name="op", bufs=3) as opool,
            tc.tile_pool(name="pp", bufs=2, space="PSUM") as ppool,
        ):
            # ---- load inputs (phase-1 dependencies first) -------------------
            enc_sb = []
            for d in range(KE):
                t_ = cpool.tile([128, TC], f32, tag=f"es{d}", name=f"es{d}")
                nc.sync.dma_start(t_[:], enc_d[d * 128:(d + 1) * 128, :])
                enc_sb.append(t_)
            w_enc_sb = []
            for d in range(KE):
                t_ = cpool.tile([128, H], f32, tag=f"we{d}", name=f"we{d}")
                nc.sync.dma_start(t_[:], w_enc_d[d * 128:(d + 1) * 128, :])
                w_enc_sb.append(t_)
            dec_sb = []
            for d in range(KD):
                t_ = cpool.tile([128, U], f32, tag=f"ds{d}", name=f"ds{d}")
                nc.sync.dma_start(t_[:], dec_d[d * 128:(d + 1) * 128, :])
                dec_sb.append(t_)
            w_pred_sb = []
            for d in range(KD):
                t_ = cpool.tile([128, H], f32, tag=f"wp{d}", name=f"wp{d}")
                nc.sync.dma_start(t_[:], w_pred_d[d * 128:(d + 1) * 128, :])
                w_pred_sb.append(t_)
            b_enc_sb = []
            b_pred_sb = []
            for k in range(KH):
                t_ = cpool.tile([128, 1], f32, tag=f"be{k}", name=f"be{k}")
                nc.sync.dma_start(t_[:], b_enc_d[k * 128:(k + 1) * 128, :])
                b_enc_sb.append(t_)
                t_ = cpool.tile([128, 1], f32, tag=f"bp{k}", name=f"bp{k}")
                nc.sync.dma_start(t_[:], b_pred_d[k * 128:(k + 1) * 128, :])
                b_pred_sb.append(t_)
            w_out_sb = []
            for k in range(KH):
                t_ = cpool.tile([128, VP], bf16, tag=f"wo{k}", name=f"wo{k}")
                nc.sync.dma_start(t_[:], w_out_d[k * 128:(k + 1) * 128, :])
                w_out_sb.append(t_)
            bout_rep = cpool.tile([128, VP], f32, tag="bo", name="bo")
            nc.sync.dma_start(bout_rep[:], b_out_d[:, :])

            # ---- phase 1: projections (full fp32) ---------------------------
            encT = []
            decT = []
            for k in range(KH):
                pe_ = ppool.tile([128, TC], f32, tag="pp1", name=f"pe{k}")
                for d in range(KE):
                    nc.tensor.matmul(
                        pe_[:], w_enc_sb[d][:, k * 128:(k + 1) * 128],
                        enc_sb[d][:], start=(d == 0), stop=(d == KE - 1))
                et = cpool.tile([128, TC], f32, tag=f"et{k}", name=f"et{k}")
                nc.scalar.add(et[:], pe_[:], b_enc_sb[k][:, 0:1])
                encT.append(et)

                pd_ = ppool.tile([128, U], f32, tag="pp1", name=f"pd{k}")
                for d in range(KD):
                    nc.tensor.matmul(
                        pd_[:], w_pred_sb[d][:, k * 128:(k + 1) * 128],
                        dec_sb[d][:], start=(d == 0), stop=(d == KD - 1))
                dtl = cpool.tile([128, U], f32, tag=f"dt{k}", name=f"dt{k}")
                nc.scalar.add(dtl[:], pd_[:], b_pred_sb[k][:, 0:1])
                decT.append(dtl)

            # ---- phase 2: joint + output matmul -----------------------------
            Relu = mybir.ActivationFunctionType.Relu
            for g in range(NG):
                # build X^T for 20 t values: one ACT Relu+bias per (t, k)
                xbs = []
                for k in range(KH):
                    xb = xpool.tile([128, GROWS], bf16, tag=f"xb{k}",
                                    name=f"xb{k}_{g}")
                    for tl in range(GT):
                        t = g * GT + tl
                        nc.scalar.activation(
                            xb[:, tl * U:(tl + 1) * U], decT[k][:],
                            Relu, bias=encT[k][:, t:t + 1], scale=1.0)
                    xbs.append(xb)

                for ci, (off, m) in enumerate(GCHUNKS):
                    r0 = g * GROWS + off
                    psums = []
                    for vi, (v0, vw) in enumerate(V_TILES):
                        ps = ppool.tile([m, vw], f32, tag=f"p{vi}",
                                        name=f"p{vi}_{g}_{ci}")
                        psums.append(ps)
                    for k in range(KH):
                        lhs = xbs[k][:, off:off + m]
                        for vi, (v0, vw) in enumerate(V_TILES):
                            nc.tensor.matmul(
                                psums[vi][:], lhs,
                                w_out_sb[k][:, v0:v0 + vw],
                                start=(k == 0), stop=(k == KH - 1))

                    osb = opool.tile([128, VP], f32, tag="osb",
                                     name=f"o{g}_{ci}")
                    for vi, (v0, vw) in enumerate(V_TILES):
                        nc.vector.tensor_add(
                            osb[0:m, v0:v0 + vw], psums[vi][:],
                            bout_rep[0:m, v0:v0 + vw])
                    nc.sync.dma_start(out_d[r0:r0 + m, :], osb[0:m, 0:V])

    nc.compile()
    return nc


def make_in_maps(inputs):
    enc = np.ascontiguousarray(np.asarray(inputs["encoder_outputs"], np.float32))
    dec = np.ascontiguousarray(np.asarray(inputs["decoder_outputs"], np.float32))
    w_enc = np.ascontiguousarray(np.asarray(inputs["W_enc"], np.float32))
    w_pred = np.ascontiguousarray(np.asarray(inputs["W_pred"], np.float32))
    import ml_dtypes
    w_out = np.zeros((H, VP), ml_dtypes.bfloat16)
    w_out[:, :V] = np.asarray(inputs["W_out"], np.float32).astype(
        ml_dtypes.bfloat16)
    b_enc = np.asarray(inputs["b_enc"], np.float32).reshape(H, 1)
    b_pred = np.asarray(inputs["b_pred"], np.float32).reshape(H, 1)
    b_out = np.zeros((1, VP), np.float32)
    b_out[0, :V] = np.asarray(inputs["b_out"], np.float32)
    b_out_rep = np.ascontiguousarray(np.broadcast_to(b_out, (128, VP)))

    in_maps = []
    for i in range(N_CORES):
        b, th = divmod(i, 2)
        in_maps.append({
            "enc": np.ascontiguousarray(enc[b, :, th * TC:(th + 1) * TC]),
            "dec": dec[b],
            "w_enc": w_enc,
            "w_pred": w_pred,
            "w_out": w_out,
            "b_enc": b_enc,
            "b_pred": b_pred,
            "b_out": b_out_rep,
        })
    return in_maps


def kernel(**inputs):
    from concourse.bass_utils import run_bass_kernel_spmd

    if "nc" not in _CACHE:
        _CACHE["nc"] = _build_program()
    nc = _CACHE["nc"]

    res = run_bass_kernel_spmd(nc, make_in_maps(inputs), list(range(N_CORES)))
    outs = np.stack([res.results[i]["out"] for i in range(N_CORES)])
    return outs.reshape(B, T, U, V).astype(np.float32, copy=False)


# revision 34
# speedup vs baseline: 1.3346x; 1.0588x over previous
"""RNN-T joint network kernel for 8 Trainium2 NeuronCores.

out[b,t,u,:] = relu(enc_proj[b,t,:] + dec_proj[b,u,:]) @ W_out + b_out

Sharding: data-parallel over (b, t-half): core i = (b = i//2, t_half = i%2)
owns output rows [i*10000, (i+1)*10000) of the flattened [80000, 1025]
output -- a contiguous 41 MB slab per core, gathered by simple concat.

Per-core plan:
  phase 1 (fp32 PE): enc_projT [H,100] and dec_projT [H,100] with bias
  phase 2: 80 chunks of 125 (t,u) rows:
    - DVE builds xT[k] = relu(decT[k][:,u-run] + encT[k][:,t]) per 128-row
      h-chunk (tensor_scalar add+max, per-partition scalar = enc column)
    - PE (float32r): psum[125, vtile] = b_out (K=1 ones matmul) then
      accumulates 5 K-chunks vs streamed W_out; V tiled {342,342,341}
    - ACT copies PSUM->SBUF, one 512 KB DMA per chunk to HBM
"""

import sys

if "/opt/trn_rl_repo" not in sys.path:
    sys.path.insert(0, "/opt/trn_rl_repo")

import numpy as np

B, D_ENC, T = 4, 512, 200
D_DEC, U = 640, 100
H, V = 640, 1025
VP = 1026                # V padded: f32r matmul needs even free counts
N_CORES = 8
TC = T // 2              # 100 t values per core
ROWS = TC * U            # 10000 output rows per core
CH = 128                 # rows per chunk (stationary M, must be even)
KE = D_ENC // 128        # 4
KD = D_DEC // 128        # 5
KH = H // 128            # 5
V_TILES = [(0, 342), (342, 342), (684, 342)]
# t-counts per X-group tile: small first groups let PE ramp into phase 2
# while later groups are still being built
GROUP_TS = [4, 8, 16, 24, 24, 24]
assert sum(GROUP_TS) == TC
N_DVE_GROUPS = 2         # first groups built on VectorE (ACT does the rest)

_CACHE = {}


def _build_program():
    import concourse.tile as tile
    from concourse import bacc, mybir

    f32 = mybir.dt.float32
    f32r = mybir.dt.float32r
    bf16 = mybir.dt.bfloat16

    nc = bacc.Bacc("TRN2", target_bir_lowering=False, debug=False,
                   num_devices=N_CORES)

    enc_d = nc.dram_tensor("enc", [D_ENC, TC], f32, kind="ExternalInput").ap()
    dec_d = nc.dram_tensor("dec", [D_DEC, U], f32, kind="ExternalInput").ap()
    w_enc_d = nc.dram_tensor("w_enc", [D_ENC, H], f32, kind="ExternalInput").ap()
    w_pred_d = nc.dram_tensor("w_pred", [D_DEC, H], f32, kind="ExternalInput").ap()
    w_out_d = nc.dram_tensor("w_out", [H, VP], bf16, kind="ExternalInput").ap()
    b_enc_d = nc.dram_tensor("b_enc", [H, 1], f32, kind="ExternalInput").ap()
    b_pred_d = nc.dram_tensor("b_pred", [H, 1], f32, kind="ExternalInput").ap()
    # b_out replicated across 128 partitions on the host
    b_out_d = nc.dram_tensor("b_out", [128, VP], f32, kind="ExternalInput").ap()
    out_d = nc.dram_tensor("out", [ROWS, V], f32, kind="ExternalOutput").ap()

    with tile.TileContext(nc) as tc:
        with (
            tc.tile_pool(name="const", bufs=1) as cpool,
            tc.tile_pool(name="xp", bufs=3) as xpool,
            tc.tile_pool(name="op", bufs=3) as opool,
            tc.tile_pool(name="pp", bufs=2, space="PSUM") as ppool,
        ):
            # ---- load inputs (phase-1 dependencies first) -------------------
            enc_sb = []
            for d in range(KE):
                t_ = cpool.tile([128, TC], f32r, tag=f"es{d}", name=f"es{d}")
                nc.sync.dma_start(
                    t_[:], enc_d[d * 128:(d + 1) * 128, :].bitcast(f32r))
                enc_sb.append(t_)
            w_enc_sb = []
            for d in range(KE):
                t_ = cpool.tile([128, H], f32r, tag=f"we{d}", name=f"we{d}")
                nc.sync.dma_start(
                    t_[:], w_enc_d[d * 128:(d + 1) * 128, :].bitcast(f32r))
                w_enc_sb.append(t_)
            dec_sb = []
            for d in range(KD):
                t_ = cpool.tile([128, U], f32r, tag=f"ds{d}", name=f"ds{d}")
                nc.sync.dma_start(
                    t_[:], dec_d[d * 128:(d + 1) * 128, :].bitcast(f32r))
                dec_sb.append(t_)
            w_pred_sb = []
            for d in range(KD):
                t_ = cpool.tile([128, H], f32r, tag=f"wp{d}", name=f"wp{d}")
                nc.sync.dma_start(
                    t_[:], w_pred_d[d * 128:(d + 1) * 128, :].bitcast(f32r))
                w_pred_sb.append(t_)
            b_enc_sb = []
            b_pred_sb = []
            for k in range(KH):
                t_ = cpool.tile([128, 1], f32, tag=f"be{k}", name=f"be{k}")
                nc.sync.dma_start(t_[:], b_enc_d[k * 128:(k + 1) * 128, :])
                b_enc_sb.append(t_)
                t_ = cpool.tile([128, 1], f32, tag=f"bp{k}", name=f"bp{k}")
                nc.sync.dma_start(t_[:], b_pred_d[k * 128:(k + 1) * 128, :])
                b_pred_sb.append(t_)
            w_out_sb = []
            for k in range(KH):
                t_ = cpool.tile([128, VP], bf16, tag=f"wo{k}", name=f"wo{k}")
                nc.sync.dma_start(t_[:], w_out_d[k * 128:(k + 1) * 128, :])
                w_out_sb.append(t_)
            bout_rep = cpool.tile([128, VP], f32, tag="bo", name="bo")
            nc.sync.dma_start(bout_rep[:], b_out_d[:, :])

            # ---- phase 1: projections (full fp32) ---------------------------
            encT = []
            decT = []
            for k in range(KH):
                pe_ = ppool.tile([128, TC], f32, tag="pp1", name=f"pe{k}")
                for d in range(KE):
                    nc.tensor.matmul(
                        pe_[:], w_enc_sb[d][:, k * 128:(k + 1) * 128],
                        enc_sb[d][:], start=(d == 0), stop=(d == KE - 1))
                et = cpool.tile([128, TC], f32, tag=f"et{k}", name=f"et{k}")
                nc.scalar.add(et[:], pe_[:], b_enc_sb[k][:, 0:1])
                encT.append(et)

                pd_ = ppool.tile([128, U], f32, tag="pp1", name=f"pd{k}")
                for d in range(KD):
                    nc.tensor.matmul(
                        pd_[:], w_pred_sb[d][:, k * 128:(k + 1) * 128],
                        dec_sb[d][:], start=(d == 0), stop=(d == KD - 1))
                dtl = cpool.tile([128, U], f32, tag=f"dt{k}", name=f"dt{k}")
                nc.scalar.add(dtl[:], pd_[:], b_pred_sb[k][:, 0:1])
                decT.append(dtl)

            # ---- phase 2: joint + output matmul -----------------------------
            Relu = mybir.ActivationFunctionType.Relu
            ADD = mybir.AluOpType.add
            MAX = mybir.AluOpType.max
            gt0 = 0
            for g, gt in enumerate(GROUP_TS):
                grows = gt * U
                # build X^T: one Relu(dec + enc[t]) per (t, k); first groups
                # go on VectorE (idle early), the rest on ScalarE
                xbs = []
                for k in range(KH):
                    xb = xpool.tile([128, grows], bf16, tag=f"xb{k}",
                                    name=f"xb{k}_{g}")
                    for tl in range(gt):
                        t = gt0 + tl
                        if g < N_DVE_GROUPS:
                            nc.vector.tensor_scalar(
                                out=xb[:, tl * U:(tl + 1) * U],
                                in0=decT[k][:],
                                scalar1=encT[k][:, t:t + 1],
                                scalar2=0.0, op0=ADD, op1=MAX)
                        else:
                            nc.scalar.activation(
                                xb[:, tl * U:(tl + 1) * U], decT[k][:],
                                Relu, bias=encT[k][:, t:t + 1], scale=1.0)
                    xbs.append(xb)

                gchunks = [(i * CH, min(CH, grows - i * CH))
                           for i in range((grows + CH - 1) // CH)]
                for ci, (off, m) in enumerate(gchunks):
                    r0 = gt0 * U + off
                    psums = []
                    for vi, (v0, vw) in enumerate(V_TILES):
                        ps = ppool.tile([m, vw], f32, tag=f"p{vi}",
                                        name=f"p{vi}_{g}_{ci}")
                        psums.append(ps)
                    for k in range(KH):
                        lhs = xbs[k][:, off:off + m]
                        for vi, (v0, vw) in enumerate(V_TILES):
                            nc.tensor.matmul(
                                psums[vi][:], lhs,
                                w_out_sb[k][:, v0:v0 + vw],
                                start=(k == 0), stop=(k == KH - 1))

                    osb = opool.tile([128, VP], f32, tag="osb",
                                     name=f"o{g}_{ci}")
                    for vi, (v0, vw) in enumerate(V_TILES):
                        nc.vector.tensor_add(
                            osb[0:m, v0:v0 + vw], psums[vi][:],
                            bout_rep[0:m, v0:v0 + vw])
                    nc.sync.dma_start(out_d[r0:r0 + m, :], osb[0:m, 0:V])
                gt0 += gt

    nc.compile()
    return nc


def make_in_maps(inputs):
    enc = np.ascontiguousarray(np.asarray(inputs["encoder_outputs"], np.float32))
    dec = np.ascontiguousarray(np.asarray(inputs["decoder_outputs"], np.float32))
    w_enc = np.ascontiguousarray(np.asarray(inputs["W_enc"], np.float32))
    w_pred = np.ascontiguousarray(np.asarray(inputs["W_pred"], np.float32))
    import ml_dtypes
    w_out = np.zeros((H, VP), ml_dtypes.bfloat16)
    w_out[:, :V] = np.asarray(inputs["W_out"], np.float32).astype(
        ml_dtypes.bfloat16)
    b_enc = np.asarray(inputs["b_enc"], np.float32).reshape(H, 1)
    b_pred = np.asarray(inputs["b_pred"], np.float32).reshape(H, 1)
    b_out = np.zeros((1, VP), np.float32)
    b_out[0, :V] = np.asarray(inputs["b_out"], np.float32)
    b_out_rep = np.ascontiguousarray(np.broadcast_to(b_out, (128, VP)))

    in_maps = []
    for i in range(N_CORES):
        b, th = divmod(i, 2)
        in_maps.append({
            "enc": np.ascontiguousarray(enc[b, :, th * TC:(th + 1) * TC]),
            "dec": dec[b],
            "w_enc": w_enc,
            "w_pred": w_pred,
            "w_out": w_out,
            "b_enc": b_enc,
            "b_pred": b_pred,
            "b_out": b_out_rep,
        })
    return in_maps


def kernel(**inputs):
    from concourse.bass_utils import run_bass_kernel_spmd

    if "nc" not in _CACHE:
        _CACHE["nc"] = _build_program()
    nc = _CACHE["nc"]

    res = run_bass_kernel_spmd(nc, make_in_maps(inputs), list(range(N_CORES)))
    outs = np.stack([res.results[i]["out"] for i in range(N_CORES)])
    return outs.reshape(B, T, U, V).astype(np.float32, copy=False)
